# revision 38
# baseline (speedup 1.0000x reference)
"""Trainium2 Bass kernel for nn_AttentionKernelIntegral (linear attention).

Math (per batch b, head h):
    q = x @ Wq^T                      [N, 512]  (no norm)
    k = inorm(x @ Wk^T)               per-(n,h) mean/var over d=64, biased
    v = inorm(x @ Wv^T)
    dots_h = k_h^T v_h                [64, 64]  (contract over ALL N)
    u_h = q_h @ dots_h / N
    out = u @ Wo^T + bo               [N, 256]

Key transforms vs the straightforward dataflow:
  * Mean subtraction folds into the weights: center Wk/Wv columns per head
    (InstanceNorm mean of a linear map = linear map with centered weights).
  * The whole q/dots/out chain folds into a per-batch [256,256] matrix:
        W3_b = (Wq/64)^T @ blockdiag(dots_b/128) @ Wo^T     (1/64*1/128=1/N)
        out  = x @ W3_b + bo
    so only k/v are ever materialized at [N,512] width.
  * rstd_k*rstd_v is computed as one rsqrt((vark+eps)*(varv+eps)) and
    applied to the k side only; v stays raw (centered) fp16.

Sharding: rows (N) split across 8 cores; only the [B,H,64,64] dots tensor
is all-reduced (fp16, scaled by 1/128 for range).
"""

import os
import sys

import numpy as np

for _p in ("/opt/trn_rl_repo", os.path.expanduser("~/.axon_site/_ro/trn_rl_repo")):
    if os.path.isdir(_p) and _p not in sys.path:
        sys.path.insert(0, _p)

from contextlib import ExitStack

import concourse.bass as bass
import concourse.mybir as mybir
import concourse.tile as tile
from concourse import bacc
from concourse.bass_utils import run_bass_kernel_spmd
from concourse.masks import make_identity

F32 = mybir.dt.float32
F16 = mybir.dt.float16

B, CIN = 4, 256
H, D = 8, 64
INNER, COUT = 512, 256
EPS = 1e-5
NCORES = 8
N_FULL = 8192
NPAIR = H // 2  # head pairs packed into 128-wide dots matmuls
W3_SCALE = 1.0 / 128.0  # wq staged as Wq*(128/N); W3 copy scales by 1/128


def _build(n_chunk, n_full=N_FULL, ncores=NCORES):
    """Build the per-core SPMD Bass program. n_chunk rows per batch per core."""
    NT = n_chunk // 128  # 128-row tiles per batch
    nc = bacc.Bacc(
        "TRN2", target_bir_lowering=False, debug=False, num_devices=ncores)

    x_d = nc.declare_dram_parameter("x", [B, n_chunk, CIN], F32, isOutput=False)
    wq_d = nc.declare_dram_parameter("wq", [INNER, CIN], F32, isOutput=False)
    wk_d = nc.declare_dram_parameter("wk", [INNER, CIN], F32, isOutput=False)
    wv_d = nc.declare_dram_parameter("wv", [INNER, CIN], F32, isOutput=False)
    wo_d = nc.declare_dram_parameter("wo", [COUT, INNER], F32, isOutput=False)
    bo_d = nc.declare_dram_parameter("bo", [1, COUT], F32, isOutput=False)
    out_d = nc.declare_dram_parameter("out", [B, n_chunk, COUT], F32, isOutput=True)

    with ExitStack() as ctx:
        tc = ctx.enter_context(tile.TileContext(nc))
        _body(ctx, tc, nc, NT, n_full, ncores,
              x_d, wq_d, wk_d, wv_d, wo_d, bo_d, out_d)
    nc.compile()
    return nc


def _body(ctx, tc, nc, NT, n_full, ncores,
          x_d, wq_d, wk_d, wv_d, wo_d, bo_d, out_d):
    n_chunk = NT * 128
    AF = mybir.ActivationFunctionType
    OP = mybir.AluOpType

    # ---------------- pools ----------------
    # PSUM: 8 banks. xpose(1) + kv(3) + dots(2) + big(2) = 8.
    xpose_ps = ctx.enter_context(tc.tile_pool(name="xpose_ps", bufs=1, space="PSUM"))
    kv_ps = ctx.enter_context(tc.tile_pool(name="kv_ps", bufs=3, space="PSUM"))
    dots_ps = ctx.enter_context(tc.tile_pool(name="dots_ps", bufs=2, space="PSUM"))
    big_ps = ctx.enter_context(tc.tile_pool(name="big_ps", bufs=2, space="PSUM"))

    consts = ctx.enter_context(tc.tile_pool(name="consts", bufs=1))
    wload = ctx.enter_context(tc.tile_pool(name="wload", bufs=2))
    xload_pool = ctx.enter_context(tc.tile_pool(name="xload", bufs=B))
    xT_pool = ctx.enter_context(tc.tile_pool(name="xT_pool", bufs=2 * B))
    kraw_pool = ctx.enter_context(tc.tile_pool(name="kraw", bufs=2 * NT))
    stats_pool = ctx.enter_context(tc.tile_pool(name="stats_pool", bufs=2))
    small_pool = ctx.enter_context(tc.tile_pool(name="small_pool", bufs=4))
    kt_pool = ctx.enter_context(tc.tile_pool(name="kt_pool", bufs=2 * NT))
    vt_pool = ctx.enter_context(tc.tile_pool(name="vt_pool", bufs=2 * NT))
    d16_pool = ctx.enter_context(tc.tile_pool(name="d16_pool", bufs=2))
    w2_pool = ctx.enter_context(tc.tile_pool(name="w2_pool", bufs=8))
    out_pool = ctx.enter_context(tc.tile_pool(name="out_pool", bufs=6))
    dram = ctx.enter_context(tc.tile_pool(name="dram", bufs=1, space="DRAM"))

    # ---------------- x loads first: plain f32 DMAs on the sync queue -------
    # (casting DMAs are slow on the fabric; cast happens in the psum->sbuf copy)
    xload = []
    for b in range(B):
        xl = xload_pool.tile([128, NT, CIN], F32, tag="xl", name=f"xload{b}")
        src = x_d[b, :, :].rearrange("(t p) c -> p t c", p=128)
        nc.sync.dma_start(xl[:], src)
        xload.append(xl)

    # ---------------- constants / weights ----------------
    ident = consts.tile([128, 128], F16, tag="ident")
    make_identity(nc, ident[:])
    ident32 = consts.tile([128, 128], F32, tag="ident32")
    make_identity(nc, ident32[:])

    # wkv_t[cs]: [128c, k(512) | v(512)] fp16, transposed from natural Wk/Wv
    wkv_t = [consts.tile([128, 2 * INNER], F16, tag=f"wkv_t{c}", name=f"wkv_t{c}")
             for c in range(2)]

    def load_transposed(w_d, n_rows, store):
        for ei in range(n_rows // 128):
            wn = wload.tile([128, CIN], F16, tag="wn")
            nc.gpsimd.dma_start(wn[:], w_d[ei * 128:(ei + 1) * 128, :])
            for cs in range(2):
                ps = xpose_ps.tile([128, 128], F16, tag="t")
                nc.tensor.transpose(ps[:], wn[:, cs * 128:(cs + 1) * 128], ident[:])
                store(ei, cs, ps)

    load_transposed(
        wk_d, INNER,
        lambda ei, cs, ps: nc.vector.tensor_copy(
            wkv_t[cs][:, ei * 128:(ei + 1) * 128], ps[:]))
    load_transposed(
        wv_d, INNER,
        lambda ei, cs, ps: nc.vector.tensor_copy(
            wkv_t[cs][:, INNER + ei * 128:INNER + (ei + 1) * 128], ps[:]))

    # center Wk/Wv per head over d (folds InstanceNorm mean into the weights)
    wkv_c = [consts.tile([128, 2 * INNER], F16, tag=f"wkv_c{c}", name=f"wkv_c{c}")
             for c in range(2)]
    for cs in range(2):
        wv_view = wkv_t[cs][:].rearrange("p (g d) -> p g d", d=D)
        msum = small_pool.tile([128, 16], F32, tag="msum")
        nc.vector.reduce_sum(msum[:], wv_view, axis=mybir.AxisListType.X)
        m16 = small_pool.tile([128, 16], F16, tag="m16")
        nc.vector.tensor_scalar_mul(m16[:], msum[:], 1.0 / D)
        nc.vector.tensor_tensor(
            wkv_c[cs][:].rearrange("p (g d) -> p g d", d=D),
            wv_view, m16[:].broadcast_to([128, 16, D]), op=OP.subtract)

    # wq natural (NOT transposed), scaled by 128/n_full: wq_nat[j] [128i, 256c]
    wq_nat = []
    for j in range(4):
        wqr = wload.tile([128, CIN], F16, tag="wqr")
        nc.gpsimd.dma_start(wqr[:], wq_d[j * 128:(j + 1) * 128, :])
        wqn = consts.tile([128, CIN], F16, tag=f"wq_nat{j}", name=f"wq_nat{j}")
        nc.vector.tensor_scalar_mul(wqn[:], wqr[:], 128.0 / n_full)
        wq_nat.append(wqn)

    # WoT: Wo [COUT, INNER] -> wo_t[j] [128i, COUT]
    wo_t = [consts.tile([128, COUT], F16, tag=f"wo_t{j}", name=f"wo_t{j}") for j in range(4)]
    for oi in range(COUT // 128):
        wn = wload.tile([128, INNER], F16, tag="wn2")
        nc.gpsimd.dma_start(wn[:], wo_d[oi * 128:(oi + 1) * 128, :])
        for j in range(4):
            ps = xpose_ps.tile([128, 128], F16, tag="t")
            nc.tensor.transpose(ps[:], wn[:, j * 128:(j + 1) * 128], ident[:])
            nc.vector.tensor_copy(wo_t[j][:, oi * 128:(oi + 1) * 128], ps[:])

    # bias broadcast [128, COUT] via ones outer product (added during out copy)
    bo_sb = consts.tile([1, COUT], F32, tag="bo_sb")
    nc.sync.dma_start(bo_sb[:], bo_d[:])
    ones1 = consts.tile([1, 128], F32, tag="ones1")
    nc.gpsimd.memset(ones1[:], 1.0)
    bias_ps = big_ps.tile([128, 512], F32, tag="t")
    nc.tensor.matmul(bias_ps[:, :COUT], ones1[:], bo_sb[:], start=True, stop=True)
    bias_bc = consts.tile([128, COUT], F32, tag="bias_bc")
    nc.vector.tensor_copy(bias_bc[:], bias_ps[:, :COUT])

    xT_all = {}     # (b, cs) -> [128c, n_chunk] f16
    kt_all = {}     # (b, nt) -> [128, 512] f16  (centered k * w)
    vt_all = {}     # (b, nt) -> [128, 512] f16  (centered v, raw)
    kraw_all = {}   # (b, nt) -> [128, 512] f16  (centered k, raw)
    bn_all = {}     # b -> [128, NT*2*8] f32 per-(row,head) sum-of-squares (k|v)
    dots16_all = {}  # b -> [128, NPAIR*64] f16 staged raw dots
    cc_out_all = {}  # b -> DRAM allreduce output

    # ------------- phase 1, split for issue-order interleaving ----------------
    def xposes(b):
        for cs in range(2):
            xT_all[(b, cs)] = xT_pool.tile([128, n_chunk], F16, tag="xT",
                                           name=f"xT_{b}_{cs}")
        for nt in range(NT):
            for cs in range(2):
                ps = xpose_ps.tile([128, 128], F32, tag="t")
                nc.tensor.transpose(
                    ps[:], xload[b][:, nt, cs * 128:(cs + 1) * 128], ident32[:])
                nc.scalar.copy(xT_all[(b, cs)][:, nt * 128:(nt + 1) * 128], ps[:])

    def kvstats(b):
        sqred = stats_pool.tile([128, NT * 16], F32, tag="sqred", name=f"sqred{b}")
        bn_all[b] = sqred
        sqredv = sqred.rearrange("p (t g h) -> p t g h", g=2, h=8)
        for nt in range(NT):
            kps = kv_ps.tile([128, 512], F32, tag="t")
            vps = kv_ps.tile([128, 512], F32, tag="t")
            for cs in range(2):
                xT_sl = xT_all[(b, cs)][:, nt * 128:(nt + 1) * 128]
                nc.tensor.matmul(kps[:], xT_sl, wkv_c[cs][:, :INNER],
                                 start=(cs == 0), stop=(cs == 1))
                nc.tensor.matmul(vps[:], xT_sl, wkv_c[cs][:, INNER:],
                                 start=(cs == 0), stop=(cs == 1))
            kr = kraw_pool.tile([128, 512], F16, tag="kr")
            vt = vt_pool.tile([128, 512], F16, tag="vt")
            kraw_all[(b, nt)] = kr
            vt_all[(b, nt)] = vt
            nc.scalar.copy(kr[:], kps[:])
            nc.scalar.copy(vt[:], vps[:])
            # squares (k on DVE f16 2x, v on Act via Square) + wide reduce (DVE)
            sq = stats_pool.tile([128, 1024], F16, tag="sq", bufs=3)
            nc.vector.tensor_tensor(sq[:, :512], kr[:], kr[:], op=OP.mult)
            nc.scalar.activation(sq[:, 512:], vt[:], AF.Square)
            nc.vector.reduce_sum(
                sqredv[:, nt, :, :],
                sq[:].rearrange("p (g h d) -> p g h d", h=8, d=D),
                axis=mybir.AxisListType.X)

    def ktdots(b):
        # batch stats: w = rstd_k*rstd_v = 1/sqrt((ksq+De)(vsq+De)/D^2)
        sqredv = bn_all[b].rearrange("p (t g h) -> p t g h", g=2, h=8)
        ksq, vsq = sqredv[:, :, 0, :], sqredv[:, :, 1, :]
        ve = small_pool.tile([128, NT * 8], F32, tag="ve")
        nc.vector.tensor_scalar_add(
            ve[:].rearrange("p (t h) -> p t h", h=8), vsq, D * EPS)
        prod = small_pool.tile([128, NT * 8], F32, tag="prod")
        nc.vector.scalar_tensor_tensor(
            prod[:].rearrange("p (t h) -> p t h", h=8),
            ksq, D * EPS, ve[:].rearrange("p (t h) -> p t h", h=8),
            op0=OP.add, op1=OP.mult)
        nc.scalar.activation(prod[:], prod[:], AF.Sqrt, scale=1.0 / (D * D))
        wsc = small_pool.tile([128, NT * 8], F32, tag="wsc")
        nc.vector.reciprocal(wsc[:], prod[:])
        w16 = small_pool.tile([128, NT * 8], F16, tag="w16")
        nc.vector.tensor_copy(w16[:], wsc[:])
        w16v = w16.rearrange("p (t h) -> p t h", h=8)

        # kt = kr * w  (DVE; the gpsimd queue must stay clear for collectives,
        # whose instruction blocks that queue until the collective completes)
        for nt in range(NT):
            kt = kt_pool.tile([128, 512], F16, tag="kt")
            kt_all[(b, nt)] = kt
            nc.vector.tensor_tensor(
                kt[:].rearrange("p (h d) -> p h d", d=D),
                kraw_all[(b, nt)][:].rearrange("p (h d) -> p h d", d=D),
                w16v[:, nt, :].broadcast_to([128, 8, D]),
                op=OP.mult)

        # dots: per head pair, accumulate kt^T vt over row tiles. Dots stay
        # UNSCALED (values ~O(100), f16-safe); 1/N folds into the W3 copy.
        bcols = NPAIR * 64
        d16 = d16_pool.tile([128, bcols], F16, tag="d16", name=f"d16_{b}")
        dots16_all[b] = d16
        for p in range(NPAIR):
            acc = dots_ps.tile([128, 128], F32, tag="t")
            for nt in range(NT):
                nc.tensor.matmul(
                    acc[:],
                    kt_all[(b, nt)][:, p * 128:(p + 1) * 128],
                    vt_all[(b, nt)][:, p * 128:(p + 1) * 128],
                    start=(nt == 0), stop=(nt == NT - 1))
            col = p * 64
            nc.scalar.copy(d16[0:64, col:col + 64], acc[0:64, 0:64])
            nc.scalar.copy(d16[64:128, col:col + 64], acc[64:128, 64:128])

        cc_in = dram.tile([128, bcols], F16, tag="cc_in", bufs=B, name=f"cc_in{b}")
        cc_out_all[b] = dram.tile([128, bcols], F16, tag="cc_out", bufs=B,
                                  name=f"cc_out{b}", addr_space="Shared")
        nc.sync.dma_start(cc_in[:], d16[:])
        nc.gpsimd.collective_compute(
            "AllReduce", OP.add,
            replica_groups=[list(range(ncores))],
            ins=[cc_in.opt()], outs=[cc_out_all[b].opt()])

    # ---------------- phase 2: compose W3 = Wq'^T dots' Wo^T, out = x W3 + bo ----
    def phase2(b):
        # fetch allreduced dots lazily (so the wait never head-of-line blocks
        # the staging of later batches' collectives)
        da = d16_pool.tile([128, NPAIR * 64], F16, tag="da", name=f"da_{b}")
        nc.sync.dma_start(da[:], cc_out_all[b][:])
        # W2T[j] [128i, 256c]: per head h, W2T rows h*64+e = dots_h^T @ Wq_h
        w2t16 = []
        for j in range(4):
            ps = big_ps.tile([128, 512], F32, tag="t")
            co = j * 64
            nc.tensor.matmul(ps[0:64, :CIN], da[0:64, co:co + 64],
                             wq_nat[j][0:64, :], start=True, stop=True)
            nc.tensor.matmul(ps[64:128, :CIN], da[64:128, co:co + 64],
                             wq_nat[j][64:128, :], start=True, stop=True)
            w2 = w2_pool.tile([128, CIN], F16, tag="w2")
            nc.scalar.copy(w2[:], ps[:, :CIN])
            w2t16.append(w2)
        # W3[cs] [128c, 256o] = sum_j W2T[j][:, cs]^T @ WoT[j]; 1/N lands here
        w3_16 = []
        for cs in range(2):
            ps = big_ps.tile([128, 512], F32, tag="t")
            for j in range(4):
                nc.tensor.matmul(ps[:, :COUT],
                                 w2t16[j][:, cs * 128:(cs + 1) * 128],
                                 wo_t[j][:], start=(j == 0), stop=(j == 3))
            w3 = w2_pool.tile([128, COUT], F16, tag="w3")
            nc.scalar.activation(w3[:], ps[:, :COUT], AF.Copy, scale=W3_SCALE)
            w3_16.append(w3)
        # out rows: out = xT^T @ W3 + bias (bias added in the psum->sbuf copy)
        for nt in range(NT):
            ops = big_ps.tile([128, 512], F32, tag="t")
            for cs in range(2):
                nc.tensor.matmul(ops[:, :COUT],
                                 xT_all[(b, cs)][:, nt * 128:(nt + 1) * 128],
                                 w3_16[cs][:], start=(cs == 0), stop=(cs == 1))
            osb = out_pool.tile([128, COUT], F32, tag="osb")
            nc.vector.tensor_tensor(osb[:], ops[:, :COUT], bias_bc[:], op=OP.add)
            nc.sync.dma_start(out_d[b, nt * 128:(nt + 1) * 128, :], osb[:])

    # schedule: PE order xp0 kv0 | xp1 dots0 kv1 | xp2 dots1 kv2 | ... so the
    # next batch's transposes fill the gap while stats/kt for batch b finish.
    xposes(0)
    kvstats(0)
    for b in range(B):
        if b + 1 < B:
            xposes(b + 1)
        ktdots(b)
        if b + 1 < B:
            kvstats(b + 1)
    for b in range(B):
        phase2(b)


_NC_CACHE = {}


def _get_nc(n_chunk, n_full, ncores):
    key = (n_chunk, n_full, ncores)
    if key not in _NC_CACHE:
        _NC_CACHE[key] = _build(n_chunk, n_full, ncores)
    return _NC_CACHE[key]


def _make_in_maps(u_x, Wq, Wk, Wv, Wo, bo, ncores):
    n = u_x.shape[1]
    n_chunk = n // ncores
    wq = np.ascontiguousarray(np.asarray(Wq, np.float32))
    wk = np.ascontiguousarray(np.asarray(Wk, np.float32))
    wv = np.ascontiguousarray(np.asarray(Wv, np.float32))
    wo = np.ascontiguousarray(np.asarray(Wo, np.float32))
    bo2 = np.ascontiguousarray(np.asarray(bo, np.float32).reshape(1, -1))
    u_x = np.asarray(u_x, np.float32)
    maps = []
    for c in range(ncores):
        maps.append({
            "x": np.ascontiguousarray(u_x[:, c * n_chunk:(c + 1) * n_chunk, :]),
            "wq": wq, "wk": wk, "wv": wv, "wo": wo, "bo": bo2,
        })
    return maps, n_chunk


def _install_ntff_hook():
    """Provide antenv.axon_hooks (missing in this image) so trace=True works."""
    import types
    try:
        from antenv.axon_hooks import get_axon_ntff_profile_hook  # noqa: F401
        return  # real module present
    except ImportError:
        pass
    try:
        import antenv
        mod = types.ModuleType("antenv.axon_hooks")
        _state = {"hook": None}
        mod.set_axon_ntff_profile_hook = lambda h: _state.__setitem__("hook", h)
        mod.get_axon_ntff_profile_hook = lambda: _state["hook"]
        sys.modules["antenv.axon_hooks"] = mod
        antenv.axon_hooks = mod
        boot_dir = "/root/.axon_site/trn_agent_boot"
        if boot_dir not in sys.path and os.path.isdir(boot_dir):
            sys.path.insert(0, boot_dir)
        import trn_boot
        so_path = "/opt/axon/libaxon_pjrt.so"
        if os.path.exists(so_path):
            hook = trn_boot._ntff_profile_via_ctypes(so_path)
            if hook is not None:
                mod.set_axon_ntff_profile_hook(hook)
    except Exception as e:  # tracing is best-effort; never break the run path
        print(f"ntff hook install failed: {e}", file=sys.stderr)


def run(u_x, Wq, Wk, Wv, Wo, bo, n_full=None, ncores=NCORES, trace=False,
        tmpdir=None):
    if trace:
        _install_ntff_hook()
    n = u_x.shape[1]
    if n_full is None:
        n_full = n
    in_maps, n_chunk = _make_in_maps(u_x, Wq, Wk, Wv, Wo, bo, ncores)
    nc = _get_nc(n_chunk, n_full, ncores)
    res = run_bass_kernel_spmd(nc, in_maps, list(range(ncores)), trace=trace,
                               tmpdir=tmpdir)
    outs = [np.asarray(res.results[c]["out"]) for c in range(ncores)]
    full = np.concatenate(outs, axis=1).astype(np.float32)
    return full, res


def kernel(u_x, pos_x=None, Wq=None, Wk=None, Wv=None, Wo=None, bo=None):
    full, _ = run(np.asarray(u_x, np.float32), Wq, Wk, Wv, Wo, bo)
    return full


# revision 39
# speedup vs baseline: 1.4214x; 1.4214x over previous
"""Trainium2 Bass kernel for nn_AttentionKernelIntegral (linear attention).

Math (per batch b, head h):
    q = x @ Wq^T                      [N, 512]  (no norm)
    k = inorm(x @ Wk^T)               per-(n,h) mean/var over d=64, biased
    v = inorm(x @ Wv^T)
    dots_h = k_h^T v_h                [64, 64]  (contract over ALL N)
    u_h = q_h @ dots_h / N
    out = u @ Wo^T + bo               [N, 256]

Key transforms vs the straightforward dataflow:
  * Mean subtraction folds into the weights: center Wk/Wv columns per head
    (InstanceNorm mean of a linear map = linear map with centered weights).
  * The whole q/dots/out chain folds into a per-batch [256,256] matrix:
        W3_b = (Wq/64)^T @ blockdiag(dots_b/128) @ Wo^T     (1/64*1/128=1/N)
        out  = x @ W3_b + bo
    so only k/v are ever materialized at [N,512] width.
  * rstd_k*rstd_v is computed as one rsqrt((vark+eps)*(varv+eps)) and
    applied to the k side only; v stays raw (centered) fp16.

Sharding: rows (N) split across 8 cores; only the [B,H,64,64] dots tensor
is all-reduced (fp16, scaled by 1/128 for range).
"""

import os
import sys

import numpy as np

for _p in ("/opt/trn_rl_repo", os.path.expanduser("~/.axon_site/_ro/trn_rl_repo")):
    if os.path.isdir(_p) and _p not in sys.path:
        sys.path.insert(0, _p)

from contextlib import ExitStack

import concourse.bass as bass
import concourse.mybir as mybir
import concourse.tile as tile
from concourse import bacc
from concourse.bass_utils import run_bass_kernel_spmd
from concourse.masks import make_identity

F32 = mybir.dt.float32
F16 = mybir.dt.float16

B, CIN = 4, 256
H, D = 8, 64
INNER, COUT = 512, 256
EPS = 1e-5
NCORES = 8
N_FULL = 8192
NPAIR = H // 2  # head pairs packed into 128-wide dots matmuls
W3_SCALE = 1.0 / 128.0  # wq staged as Wq*(128/N); W3 copy scales by 1/128


def _build(n_chunk, n_full=N_FULL, ncores=NCORES):
    """Build the per-core SPMD Bass program. n_chunk rows per batch per core."""
    NT = n_chunk // 128  # 128-row tiles per batch
    nc = bacc.Bacc(
        "TRN2", target_bir_lowering=False, debug=False, num_devices=ncores)

    x_d = nc.declare_dram_parameter("x", [B, n_chunk, CIN], F32, isOutput=False)
    wq_d = nc.declare_dram_parameter("wq", [INNER, CIN], F32, isOutput=False)
    wk_d = nc.declare_dram_parameter("wk", [INNER, CIN], F32, isOutput=False)
    wv_d = nc.declare_dram_parameter("wv", [INNER, CIN], F32, isOutput=False)
    wo_d = nc.declare_dram_parameter("wo", [COUT, INNER], F32, isOutput=False)
    bo_d = nc.declare_dram_parameter("bo", [1, COUT], F32, isOutput=False)
    out_d = nc.declare_dram_parameter("out", [B, n_chunk, COUT], F32, isOutput=True)

    with ExitStack() as ctx:
        tc = ctx.enter_context(tile.TileContext(nc))
        _body(ctx, tc, nc, NT, n_full, ncores,
              x_d, wq_d, wk_d, wv_d, wo_d, bo_d, out_d)
    nc.compile()
    return nc


def _body(ctx, tc, nc, NT, n_full, ncores,
          x_d, wq_d, wk_d, wv_d, wo_d, bo_d, out_d):
    n_chunk = NT * 128
    AF = mybir.ActivationFunctionType
    OP = mybir.AluOpType

    # ---------------- pools ----------------
    # PSUM: 8 banks. xpose(1) + kv(3) + dots(2) + big(2) = 8.
    xpose_ps = ctx.enter_context(tc.tile_pool(name="xpose_ps", bufs=1, space="PSUM"))
    kv_ps = ctx.enter_context(tc.tile_pool(name="kv_ps", bufs=3, space="PSUM"))
    dots_ps = ctx.enter_context(tc.tile_pool(name="dots_ps", bufs=2, space="PSUM"))
    big_ps = ctx.enter_context(tc.tile_pool(name="big_ps", bufs=2, space="PSUM"))

    consts = ctx.enter_context(tc.tile_pool(name="consts", bufs=1))
    wload = ctx.enter_context(tc.tile_pool(name="wload", bufs=2))
    xload_pool = ctx.enter_context(tc.tile_pool(name="xload", bufs=B))
    xT_pool = ctx.enter_context(tc.tile_pool(name="xT_pool", bufs=2 * B))
    kraw_pool = ctx.enter_context(tc.tile_pool(name="kraw", bufs=2 * NT))
    stats_pool = ctx.enter_context(tc.tile_pool(name="stats_pool", bufs=2))
    small_pool = ctx.enter_context(tc.tile_pool(name="small_pool", bufs=4))
    kt_pool = ctx.enter_context(tc.tile_pool(name="kt_pool", bufs=2 * NT))
    vt_pool = ctx.enter_context(tc.tile_pool(name="vt_pool", bufs=2 * NT))
    d16_pool = ctx.enter_context(tc.tile_pool(name="d16_pool", bufs=2))
    w2_pool = ctx.enter_context(tc.tile_pool(name="w2_pool", bufs=8))
    out_pool = ctx.enter_context(tc.tile_pool(name="out_pool", bufs=6))
    dram = ctx.enter_context(tc.tile_pool(name="dram", bufs=1, space="DRAM"))

    # ---------------- x loads first: plain f32 DMAs on the sync queue -------
    # (casting DMAs are slow on the fabric; cast happens in the psum->sbuf copy)
    xload = []
    for b in range(B):
        xl = xload_pool.tile([128, NT, CIN], F32, tag="xl", name=f"xload{b}")
        src = x_d[b, :, :].rearrange("(t p) c -> p t c", p=128)
        nc.sync.dma_start(xl[:], src)
        xload.append(xl)

    # ---------------- constants / weights ----------------
    ident = consts.tile([128, 128], F16, tag="ident")
    make_identity(nc, ident[:])
    ident32 = consts.tile([128, 128], F32, tag="ident32")
    make_identity(nc, ident32[:])

    # wkv_t[cs]: [128c, k(512) | v(512)] fp16, transposed from natural Wk/Wv
    wkv_t = [consts.tile([128, 2 * INNER], F16, tag=f"wkv_t{c}", name=f"wkv_t{c}")
             for c in range(2)]

    def load_transposed(w_d, n_rows, store):
        for ei in range(n_rows // 128):
            wn = wload.tile([128, CIN], F16, tag="wn")
            nc.gpsimd.dma_start(wn[:], w_d[ei * 128:(ei + 1) * 128, :])
            for cs in range(2):
                ps = xpose_ps.tile([128, 128], F16, tag="t")
                nc.tensor.transpose(ps[:], wn[:, cs * 128:(cs + 1) * 128], ident[:])
                store(ei, cs, ps)

    load_transposed(
        wk_d, INNER,
        lambda ei, cs, ps: nc.scalar.copy(wkv_t[cs][:, ei * 128:(ei + 1) * 128], ps[:]))
    load_transposed(
        wv_d, INNER,
        lambda ei, cs, ps: nc.scalar.copy(
            wkv_t[cs][:, INNER + ei * 128:INNER + (ei + 1) * 128], ps[:]))

    # center Wk/Wv per head over d (folds InstanceNorm mean into the weights)
    wkv_c = [consts.tile([128, 2 * INNER], F16, tag=f"wkv_c{c}", name=f"wkv_c{c}")
             for c in range(2)]
    for cs in range(2):
        wv_view = wkv_t[cs][:].rearrange("p (g d) -> p g d", d=D)
        msum = small_pool.tile([128, 16], F32, tag="msum")
        nc.vector.reduce_sum(msum[:], wv_view, axis=mybir.AxisListType.X)
        m16 = small_pool.tile([128, 16], F16, tag="m16")
        nc.vector.tensor_scalar_mul(m16[:], msum[:], 1.0 / D)
        nc.vector.tensor_tensor(
            wkv_c[cs][:].rearrange("p (g d) -> p g d", d=D),
            wv_view, m16[:].broadcast_to([128, 16, D]), op=OP.subtract)

    # wq natural (NOT transposed), scaled by 128/n_full: wq_nat[j] [128i, 256c]
    wq_nat = []
    for j in range(4):
        wqr = wload.tile([128, CIN], F16, tag="wqr")
        nc.gpsimd.dma_start(wqr[:], wq_d[j * 128:(j + 1) * 128, :])
        wqn = consts.tile([128, CIN], F16, tag=f"wq_nat{j}", name=f"wq_nat{j}")
        nc.scalar.activation(wqn[:], wqr[:], AF.Copy, scale=128.0 / n_full)
        wq_nat.append(wqn)

    # WoT: Wo [COUT, INNER] -> wo_t[j] [128i, COUT]
    wo_t = [consts.tile([128, COUT], F16, tag=f"wo_t{j}", name=f"wo_t{j}") for j in range(4)]
    for oi in range(COUT // 128):
        wn = wload.tile([128, INNER], F16, tag="wn2")
        nc.gpsimd.dma_start(wn[:], wo_d[oi * 128:(oi + 1) * 128, :])
        for j in range(4):
            ps = xpose_ps.tile([128, 128], F16, tag="t")
            nc.tensor.transpose(ps[:], wn[:, j * 128:(j + 1) * 128], ident[:])
            nc.scalar.copy(wo_t[j][:, oi * 128:(oi + 1) * 128], ps[:])

    # bias broadcast [128, COUT] via ones outer product (added during out copy)
    bo_sb = consts.tile([1, COUT], F32, tag="bo_sb")
    nc.sync.dma_start(bo_sb[:], bo_d[:])
    ones1 = consts.tile([1, 128], F32, tag="ones1")
    nc.gpsimd.memset(ones1[:], 1.0)
    bias_ps = big_ps.tile([128, 512], F32, tag="t")
    nc.tensor.matmul(bias_ps[:, :COUT], ones1[:], bo_sb[:], start=True, stop=True)
    bias_bc = consts.tile([128, COUT], F32, tag="bias_bc")
    nc.scalar.copy(bias_bc[:], bias_ps[:, :COUT])

    xT_all = {}     # (b, cs) -> [128c, n_chunk] f16
    kt_all = {}     # (b, nt) -> [128, 512] f16  (centered k * w)
    vt_all = {}     # (b, nt) -> [128, 512] f16  (centered v, raw)
    kraw_all = {}   # (b, nt) -> [128, 512] f16  (centered k, raw)
    bn_all = {}     # b -> [128, NT*2*8] f32 per-(row,head) sum-of-squares (k|v)
    dots16_all = {}  # b -> [128, NPAIR*64] f16 staged raw dots
    cc_out_all = {}  # b -> DRAM allreduce output

    # ------------- phase 1, split for issue-order interleaving ----------------
    def xposes(b):
        for cs in range(2):
            xT_all[(b, cs)] = xT_pool.tile([128, n_chunk], F16, tag="xT",
                                           name=f"xT_{b}_{cs}")
        for nt in range(NT):
            for cs in range(2):
                ps = xpose_ps.tile([128, 128], F32, tag="t")
                nc.tensor.transpose(
                    ps[:], xload[b][:, nt, cs * 128:(cs + 1) * 128], ident32[:])
                nc.scalar.copy(xT_all[(b, cs)][:, nt * 128:(nt + 1) * 128], ps[:])

    def kvstats(b):
        sqred = stats_pool.tile([128, NT * 16], F32, tag="sqred", name=f"sqred{b}")
        bn_all[b] = sqred
        sqredv = sqred.rearrange("p (t g h) -> p t g h", g=2, h=8)
        for nt in range(NT):
            kps = kv_ps.tile([128, 512], F32, tag="t")
            vps = kv_ps.tile([128, 512], F32, tag="t")
            for cs in range(2):
                xT_sl = xT_all[(b, cs)][:, nt * 128:(nt + 1) * 128]
                nc.tensor.matmul(kps[:], xT_sl, wkv_c[cs][:, :INNER],
                                 start=(cs == 0), stop=(cs == 1))
                nc.tensor.matmul(vps[:], xT_sl, wkv_c[cs][:, INNER:],
                                 start=(cs == 0), stop=(cs == 1))
            kr = kraw_pool.tile([128, 512], F16, tag="kr")
            vt = vt_pool.tile([128, 512], F16, tag="vt")
            kraw_all[(b, nt)] = kr
            vt_all[(b, nt)] = vt
            nc.scalar.copy(kr[:], kps[:])
            nc.scalar.copy(vt[:], vps[:])
            # squares (k on DVE f16 2x, v on Act via Square) + wide reduce (DVE)
            sq = stats_pool.tile([128, 1024], F16, tag="sq", bufs=3)
            nc.vector.tensor_tensor(sq[:, :512], kr[:], kr[:], op=OP.mult)
            nc.scalar.activation(sq[:, 512:], vt[:], AF.Square)
            nc.vector.reduce_sum(
                sqredv[:, nt, :, :],
                sq[:].rearrange("p (g h d) -> p g h d", h=8, d=D),
                axis=mybir.AxisListType.X)

    def ktdots(b):
        # batch stats: w = rstd_k*rstd_v = 1/sqrt((ksq+De)(vsq+De)/D^2)
        sqredv = bn_all[b].rearrange("p (t g h) -> p t g h", g=2, h=8)
        ksq, vsq = sqredv[:, :, 0, :], sqredv[:, :, 1, :]
        ve = small_pool.tile([128, NT * 8], F32, tag="ve")
        nc.vector.tensor_scalar_add(
            ve[:].rearrange("p (t h) -> p t h", h=8), vsq, D * EPS)
        prod = small_pool.tile([128, NT * 8], F32, tag="prod")
        nc.vector.scalar_tensor_tensor(
            prod[:].rearrange("p (t h) -> p t h", h=8),
            ksq, D * EPS, ve[:].rearrange("p (t h) -> p t h", h=8),
            op0=OP.add, op1=OP.mult)
        nc.scalar.activation(prod[:], prod[:], AF.Sqrt, scale=1.0 / (D * D))
        wsc = small_pool.tile([128, NT * 8], F32, tag="wsc")
        nc.vector.reciprocal(wsc[:], prod[:])
        w16 = small_pool.tile([128, NT * 8], F16, tag="w16")
        nc.vector.tensor_copy(w16[:], wsc[:])
        w16v = w16.rearrange("p (t h) -> p t h", h=8)

        # kt = kr * w  (DVE; the gpsimd queue must stay clear for collectives,
        # whose instruction blocks that queue until the collective completes)
        for nt in range(NT):
            kt = kt_pool.tile([128, 512], F16, tag="kt")
            kt_all[(b, nt)] = kt
            nc.vector.tensor_tensor(
                kt[:].rearrange("p (h d) -> p h d", d=D),
                kraw_all[(b, nt)][:].rearrange("p (h d) -> p h d", d=D),
                w16v[:, nt, :].broadcast_to([128, 8, D]),
                op=OP.mult)

        # dots: per head pair, accumulate kt^T vt over row tiles. Dots stay
        # UNSCALED (values ~O(100), f16-safe); 1/N folds into the W3 copy.
        bcols = NPAIR * 64
        d16 = d16_pool.tile([128, bcols], F16, tag="d16", name=f"d16_{b}")
        dots16_all[b] = d16
        for p in range(NPAIR):
            acc = dots_ps.tile([128, 128], F32, tag="t")
            for nt in range(NT):
                nc.tensor.matmul(
                    acc[:],
                    kt_all[(b, nt)][:, p * 128:(p + 1) * 128],
                    vt_all[(b, nt)][:, p * 128:(p + 1) * 128],
                    start=(nt == 0), stop=(nt == NT - 1))
            col = p * 64
            nc.scalar.copy(d16[0:64, col:col + 64], acc[0:64, 0:64])
            nc.scalar.copy(d16[64:128, col:col + 64], acc[64:128, 64:128])

        cc_in = dram.tile([128, bcols], F16, tag="cc_in", bufs=B, name=f"cc_in{b}")
        cc_out_all[b] = dram.tile([128, bcols], F16, tag="cc_out", bufs=B,
                                  name=f"cc_out{b}", addr_space="Shared")
        nc.sync.dma_start(cc_in[:], d16[:])
        nc.gpsimd.collective_compute(
            "AllReduce", OP.add,
            replica_groups=[list(range(ncores))],
            ins=[cc_in.opt()], outs=[cc_out_all[b].opt()])

    # ---------------- phase 2: compose W3 = Wq'^T dots' Wo^T, out = x W3 + bo ----
    def phase2(b):
        # fetch allreduced dots lazily (so the wait never head-of-line blocks
        # the staging of later batches' collectives)
        da = d16_pool.tile([128, NPAIR * 64], F16, tag="da", name=f"da_{b}")
        nc.sync.dma_start(da[:], cc_out_all[b][:])
        # W2T[j] [128i, 256c]: per head h, W2T rows h*64+e = dots_h^T @ Wq_h
        w2t16 = []
        for j in range(4):
            ps = big_ps.tile([128, 512], F32, tag="t")
            co = j * 64
            nc.tensor.matmul(ps[0:64, :CIN], da[0:64, co:co + 64],
                             wq_nat[j][0:64, :], start=True, stop=True)
            nc.tensor.matmul(ps[64:128, :CIN], da[64:128, co:co + 64],
                             wq_nat[j][64:128, :], start=True, stop=True)
            w2 = w2_pool.tile([128, CIN], F16, tag="w2")
            nc.scalar.copy(w2[:], ps[:, :CIN])
            w2t16.append(w2)
        # W3[cs] [128c, 256o] = sum_j W2T[j][:, cs]^T @ WoT[j]; 1/N lands here
        w3_16 = []
        for cs in range(2):
            ps = big_ps.tile([128, 512], F32, tag="t")
            for j in range(4):
                nc.tensor.matmul(ps[:, :COUT],
                                 w2t16[j][:, cs * 128:(cs + 1) * 128],
                                 wo_t[j][:], start=(j == 0), stop=(j == 3))
            w3 = w2_pool.tile([128, COUT], F16, tag="w3")
            nc.scalar.activation(w3[:], ps[:, :COUT], AF.Copy, scale=W3_SCALE)
            w3_16.append(w3)
        # out rows: out = xT^T @ W3 + bias (bias added in the psum->sbuf copy)
        for nt in range(NT):
            ops = big_ps.tile([128, 512], F32, tag="t")
            for cs in range(2):
                nc.tensor.matmul(ops[:, :COUT],
                                 xT_all[(b, cs)][:, nt * 128:(nt + 1) * 128],
                                 w3_16[cs][:], start=(cs == 0), stop=(cs == 1))
            osb = out_pool.tile([128, COUT], F32, tag="osb")
            nc.vector.tensor_tensor(osb[:], ops[:, :COUT], bias_bc[:], op=OP.add)
            nc.sync.dma_start(out_d[b, nt * 128:(nt + 1) * 128, :], osb[:])

    # schedule: PE order xp0 kv0 | xp1 dots0 kv1 | xp2 dots1 kv2 | ... so the
    # next batch's transposes fill the gap while stats/kt for batch b finish.
    xposes(0)
    kvstats(0)
    for b in range(B):
        if b + 1 < B:
            xposes(b + 1)
        if b == B - 1:
            phase2(0)
        ktdots(b)
        if b + 1 < B:
            kvstats(b + 1)
    for b in range(1, B):
        phase2(b)


_NC_CACHE = {}


def _get_nc(n_chunk, n_full, ncores):
    key = (n_chunk, n_full, ncores)
    if key not in _NC_CACHE:
        _NC_CACHE[key] = _build(n_chunk, n_full, ncores)
    return _NC_CACHE[key]


def _make_in_maps(u_x, Wq, Wk, Wv, Wo, bo, ncores):
    n = u_x.shape[1]
    n_chunk = n // ncores
    wq = np.ascontiguousarray(np.asarray(Wq, np.float32))
    wk = np.ascontiguousarray(np.asarray(Wk, np.float32))
    wv = np.ascontiguousarray(np.asarray(Wv, np.float32))
    wo = np.ascontiguousarray(np.asarray(Wo, np.float32))
    bo2 = np.ascontiguousarray(np.asarray(bo, np.float32).reshape(1, -1))
    u_x = np.asarray(u_x, np.float32)
    maps = []
    for c in range(ncores):
        maps.append({
            "x": np.ascontiguousarray(u_x[:, c * n_chunk:(c + 1) * n_chunk, :]),
            "wq": wq, "wk": wk, "wv": wv, "wo": wo, "bo": bo2,
        })
    return maps, n_chunk


def _install_ntff_hook():
    """Provide antenv.axon_hooks (missing in this image) so trace=True works."""
    import types
    try:
        from antenv.axon_hooks import get_axon_ntff_profile_hook  # noqa: F401
        return  # real module present
    except ImportError:
        pass
    try:
        import antenv
        mod = types.ModuleType("antenv.axon_hooks")
        _state = {"hook": None}
        mod.set_axon_ntff_profile_hook = lambda h: _state.__setitem__("hook", h)
        mod.get_axon_ntff_profile_hook = lambda: _state["hook"]
        sys.modules["antenv.axon_hooks"] = mod
        antenv.axon_hooks = mod
        boot_dir = "/root/.axon_site/trn_agent_boot"
        if boot_dir not in sys.path and os.path.isdir(boot_dir):
            sys.path.insert(0, boot_dir)
        import trn_boot
        so_path = "/opt/axon/libaxon_pjrt.so"
        if os.path.exists(so_path):
            hook = trn_boot._ntff_profile_via_ctypes(so_path)
            if hook is not None:
                mod.set_axon_ntff_profile_hook(hook)
    except Exception as e:  # tracing is best-effort; never break the run path
        print(f"ntff hook install failed: {e}", file=sys.stderr)


def run(u_x, Wq, Wk, Wv, Wo, bo, n_full=None, ncores=NCORES, trace=False,
        tmpdir=None):
    if trace:
        _install_ntff_hook()
    n = u_x.shape[1]
    if n_full is None:
        n_full = n
    in_maps, n_chunk = _make_in_maps(u_x, Wq, Wk, Wv, Wo, bo, ncores)
    nc = _get_nc(n_chunk, n_full, ncores)
    res = run_bass_kernel_spmd(nc, in_maps, list(range(ncores)), trace=trace,
                               tmpdir=tmpdir)
    outs = [np.asarray(res.results[c]["out"]) for c in range(ncores)]
    full = np.concatenate(outs, axis=1).astype(np.float32)
    return full, res


def kernel(u_x, pos_x=None, Wq=None, Wk=None, Wv=None, Wo=None, bo=None):
    full, _ = run(np.asarray(u_x, np.float32), Wq, Wk, Wv, Wo, bo)
    return full


# revision 42
# speedup vs baseline: 1.6979x; 1.1945x over previous
"""Trainium2 Bass kernel for nn_AttentionKernelIntegral (linear attention).

Math (per batch b, head h):
    q = x @ Wq^T                      [N, 512]  (no norm)
    k = inorm(x @ Wk^T)               per-(n,h) mean/var over d=64, biased
    v = inorm(x @ Wv^T)
    dots_h = k_h^T v_h                [64, 64]  (contract over ALL N)
    u_h = q_h @ dots_h / N
    out = u @ Wo^T + bo               [N, 256]

Key transforms vs the straightforward dataflow:
  * Mean subtraction folds into the weights: center Wk/Wv columns per head
    (InstanceNorm mean of a linear map = linear map with centered weights).
  * The whole q/dots/out chain folds into a per-batch [256,256] matrix:
        W3_b = (Wq/64)^T @ blockdiag(dots_b/128) @ Wo^T     (1/64*1/128=1/N)
        out  = x @ W3_b + bo
    so only k/v are ever materialized at [N,512] width.
  * rstd_k*rstd_v is computed as one rsqrt((vark+eps)*(varv+eps)) and
    applied to the k side only; v stays raw (centered) fp16.

Sharding: rows (N) split across 8 cores; only the [B,H,64,64] dots tensor
is all-reduced (fp16, scaled by 1/128 for range).
"""

import os
import sys

import numpy as np

for _p in ("/opt/trn_rl_repo", os.path.expanduser("~/.axon_site/_ro/trn_rl_repo")):
    if os.path.isdir(_p) and _p not in sys.path:
        sys.path.insert(0, _p)

from contextlib import ExitStack

import concourse.bass as bass
import concourse.mybir as mybir
import concourse.tile as tile
from concourse import bacc
from concourse.bass_utils import run_bass_kernel_spmd
from concourse.masks import make_identity

F32 = mybir.dt.float32
F16 = mybir.dt.float16

B, CIN = 4, 256
H, D = 8, 64
INNER, COUT = 512, 256
EPS = 1e-5
NCORES = 8
N_FULL = 8192
NPAIR = H // 2  # head pairs packed into 128-wide dots matmuls
W3_SCALE = 1.0 / 128.0  # wq staged as Wq*(128/N); W3 copy scales by 1/128


def _build(n_chunk, n_full=N_FULL, ncores=NCORES):
    """Build the per-core SPMD Bass program. n_chunk rows per batch per core."""
    NT = n_chunk // 128  # 128-row tiles per batch
    nc = bacc.Bacc(
        "TRN2", target_bir_lowering=False, debug=False, num_devices=ncores)

    x_d = nc.declare_dram_parameter("x", [B, n_chunk, CIN], F32, isOutput=False)
    wq_d = nc.declare_dram_parameter("wq", [INNER, CIN], F32, isOutput=False)
    wk_d = nc.declare_dram_parameter("wk", [INNER, CIN], F32, isOutput=False)
    wv_d = nc.declare_dram_parameter("wv", [INNER, CIN], F32, isOutput=False)
    wo_d = nc.declare_dram_parameter("wo", [COUT, INNER], F32, isOutput=False)
    bo_d = nc.declare_dram_parameter("bo", [1, COUT], F32, isOutput=False)
    out_d = nc.declare_dram_parameter("out", [B, n_chunk, COUT], F32, isOutput=True)

    with ExitStack() as ctx:
        tc = ctx.enter_context(tile.TileContext(nc))
        _body(ctx, tc, nc, NT, n_full, ncores,
              x_d, wq_d, wk_d, wv_d, wo_d, bo_d, out_d)
    nc.compile()
    return nc


def _body(ctx, tc, nc, NT, n_full, ncores,
          x_d, wq_d, wk_d, wv_d, wo_d, bo_d, out_d):
    n_chunk = NT * 128
    AF = mybir.ActivationFunctionType
    OP = mybir.AluOpType

    # ---------------- pools ----------------
    # PSUM: 8 banks. xpose(1) + kv(3) + dots(2) + big(2) = 8.
    xpose_ps = ctx.enter_context(tc.tile_pool(name="xpose_ps", bufs=1, space="PSUM"))
    kv_ps = ctx.enter_context(tc.tile_pool(name="kv_ps", bufs=3, space="PSUM"))
    dots_ps = ctx.enter_context(tc.tile_pool(name="dots_ps", bufs=2, space="PSUM"))
    big_ps = ctx.enter_context(tc.tile_pool(name="big_ps", bufs=2, space="PSUM"))

    consts = ctx.enter_context(tc.tile_pool(name="consts", bufs=1))
    wload = ctx.enter_context(tc.tile_pool(name="wload", bufs=2))
    xload_pool = ctx.enter_context(tc.tile_pool(name="xload", bufs=B))
    xT_pool = ctx.enter_context(tc.tile_pool(name="xT_pool", bufs=2 * B))
    kraw_pool = ctx.enter_context(tc.tile_pool(name="kraw", bufs=2 * NT))
    stats_pool = ctx.enter_context(tc.tile_pool(name="stats_pool", bufs=2))
    small_pool = ctx.enter_context(tc.tile_pool(name="small_pool", bufs=4))
    kt_pool = ctx.enter_context(tc.tile_pool(name="kt_pool", bufs=2 * NT))
    vt_pool = ctx.enter_context(tc.tile_pool(name="vt_pool", bufs=2 * NT))
    d16_pool = ctx.enter_context(tc.tile_pool(name="d16_pool", bufs=2))
    w2_pool = ctx.enter_context(tc.tile_pool(name="w2_pool", bufs=8))
    out_pool = ctx.enter_context(tc.tile_pool(name="out_pool", bufs=6))
    dram = ctx.enter_context(tc.tile_pool(name="dram", bufs=1, space="DRAM"))

    # ---------------- x loads first: plain f32 DMAs on the sync queue -------
    # (casting DMAs are slow on the fabric; cast happens in the psum->sbuf copy)
    xload = []
    for b in range(B):
        xl = xload_pool.tile([128, NT, CIN], F32, tag="xl", name=f"xload{b}")
        src = x_d[b, :, :].rearrange("(t p) c -> p t c", p=128)
        nc.sync.dma_start(xl[:], src)
        xload.append(xl)

    # ---------------- constants / weights ----------------
    ident = consts.tile([128, 128], F16, tag="ident")
    make_identity(nc, ident[:])
    ident32 = consts.tile([128, 128], F32, tag="ident32")
    make_identity(nc, ident32[:])

    # wkv_t[cs]: [128c, k(512) | v(512)] fp16, transposed from natural Wk/Wv
    wkv_t = [consts.tile([128, 2 * INNER], F16, tag=f"wkv_t{c}", name=f"wkv_t{c}")
             for c in range(2)]

    def load_transposed(w_d, n_rows, store):
        for ei in range(n_rows // 128):
            wn = wload.tile([128, CIN], F16, tag="wn")
            nc.gpsimd.dma_start(wn[:], w_d[ei * 128:(ei + 1) * 128, :])
            for cs in range(2):
                ps = xpose_ps.tile([128, 128], F16, tag="t")
                nc.tensor.transpose(ps[:], wn[:, cs * 128:(cs + 1) * 128], ident[:])
                store(ei, cs, ps)

    load_transposed(
        wk_d, INNER,
        lambda ei, cs, ps: nc.scalar.copy(wkv_t[cs][:, ei * 128:(ei + 1) * 128], ps[:]))
    load_transposed(
        wv_d, INNER,
        lambda ei, cs, ps: nc.scalar.copy(
            wkv_t[cs][:, INNER + ei * 128:INNER + (ei + 1) * 128], ps[:]))

    # center Wk/Wv per head over d (folds InstanceNorm mean into the weights)
    wkv_c = [consts.tile([128, 2 * INNER], F16, tag=f"wkv_c{c}", name=f"wkv_c{c}")
             for c in range(2)]
    for cs in range(2):
        wv_view = wkv_t[cs][:].rearrange("p (g d) -> p g d", d=D)
        msum = small_pool.tile([128, 16], F32, tag="msum")
        nc.vector.reduce_sum(msum[:], wv_view, axis=mybir.AxisListType.X)
        m16 = small_pool.tile([128, 16], F16, tag="m16")
        nc.vector.tensor_scalar_mul(m16[:], msum[:], 1.0 / D)
        nc.vector.tensor_tensor(
            wkv_c[cs][:].rearrange("p (g d) -> p g d", d=D),
            wv_view, m16[:].broadcast_to([128, 16, D]), op=OP.subtract)

    # wq natural (NOT transposed), scaled by 128/n_full: wq_nat[j] [128i, 256c]
    wq_nat = []
    for j in range(4):
        wqr = wload.tile([128, CIN], F16, tag="wqr")
        nc.gpsimd.dma_start(wqr[:], wq_d[j * 128:(j + 1) * 128, :])
        wqn = consts.tile([128, CIN], F16, tag=f"wq_nat{j}", name=f"wq_nat{j}")
        nc.scalar.activation(wqn[:], wqr[:], AF.Copy, scale=128.0 / n_full)
        wq_nat.append(wqn)

    # WoT: Wo [COUT, INNER] -> wo_t[j] [128i, COUT]
    wo_t = [consts.tile([128, COUT], F16, tag=f"wo_t{j}", name=f"wo_t{j}") for j in range(4)]
    for oi in range(COUT // 128):
        wn = wload.tile([128, INNER], F16, tag="wn2")
        nc.gpsimd.dma_start(wn[:], wo_d[oi * 128:(oi + 1) * 128, :])
        for j in range(4):
            ps = xpose_ps.tile([128, 128], F16, tag="t")
            nc.tensor.transpose(ps[:], wn[:, j * 128:(j + 1) * 128], ident[:])
            nc.scalar.copy(wo_t[j][:, oi * 128:(oi + 1) * 128], ps[:])

    # bias broadcast [128, COUT] via ones outer product (added during out copy)
    bo_sb = consts.tile([1, COUT], F32, tag="bo_sb")
    nc.sync.dma_start(bo_sb[:], bo_d[:])
    ones1 = consts.tile([1, 128], F32, tag="ones1")
    nc.gpsimd.memset(ones1[:], 1.0)
    bias_ps = big_ps.tile([128, 512], F32, tag="t")
    nc.tensor.matmul(bias_ps[:, :COUT], ones1[:], bo_sb[:], start=True, stop=True)
    bias_bc = consts.tile([128, COUT], F32, tag="bias_bc")
    nc.scalar.copy(bias_bc[:], bias_ps[:, :COUT])

    xT_all = {}     # (b, cs) -> [128c, n_chunk] f16
    kt_all = {}     # (b, nt) -> [128, 512] f16  (centered k * w)
    vt_all = {}     # (b, nt) -> [128, 512] f16  (centered v, raw)
    kraw_all = {}   # (b, nt) -> [128, 512] f16  (centered k, raw)
    bn_all = {}     # b -> [128, NT*2*8] f32 per-(row,head) sum-of-squares (k|v)
    dots16_all = {}  # b -> [128, NPAIR*64] f16 staged raw dots
    cc_out_all = {}  # b -> DRAM allreduce output

    # ------------- phase 1, split for issue-order interleaving ----------------
    def xposes(b):
        for cs in range(2):
            xT_all[(b, cs)] = xT_pool.tile([128, n_chunk], F16, tag="xT",
                                           name=f"xT_{b}_{cs}")
        for nt in range(NT):
            for cs in range(2):
                ps = xpose_ps.tile([128, 128], F32, tag="t")
                nc.tensor.transpose(
                    ps[:], xload[b][:, nt, cs * 128:(cs + 1) * 128], ident32[:])
                nc.scalar.copy(xT_all[(b, cs)][:, nt * 128:(nt + 1) * 128], ps[:])

    def kvstats(b):
        sqred = stats_pool.tile([128, NT * 16], F32, tag="sqred", name=f"sqred{b}")
        bn_all[b] = sqred
        sqredv = sqred.rearrange("p (t g h) -> p t g h", g=2, h=8)
        for nt in range(NT):
            kps = kv_ps.tile([128, 512], F32, tag="t")
            vps = kv_ps.tile([128, 512], F32, tag="t")
            for cs in range(2):
                xT_sl = xT_all[(b, cs)][:, nt * 128:(nt + 1) * 128]
                nc.tensor.matmul(kps[:], xT_sl, wkv_c[cs][:, :INNER],
                                 start=(cs == 0), stop=(cs == 1))
                nc.tensor.matmul(vps[:], xT_sl, wkv_c[cs][:, INNER:],
                                 start=(cs == 0), stop=(cs == 1))
            kr = kraw_pool.tile([128, 512], F16, tag="kr")
            vt = vt_pool.tile([128, 512], F16, tag="vt")
            kraw_all[(b, nt)] = kr
            vt_all[(b, nt)] = vt
            nc.scalar.copy(kr[:], kps[:])
            nc.scalar.copy(vt[:], vps[:])
            # squares (k on DVE f16 2x, v on Act via Square) + wide reduce (DVE)
            sq = stats_pool.tile([128, 1024], F16, tag="sq", bufs=3)
            nc.vector.tensor_tensor(sq[:, :512], kr[:], kr[:], op=OP.mult)
            nc.scalar.activation(sq[:, 512:], vt[:], AF.Square)
            nc.vector.reduce_sum(
                sqredv[:, nt, :, :],
                sq[:].rearrange("p (g h d) -> p g h d", h=8, d=D),
                axis=mybir.AxisListType.X)

    def ktdots(b):
        # batch stats: w = rstd_k*rstd_v = 1/sqrt((ksq+De)(vsq+De)/D^2)
        sqredv = bn_all[b].rearrange("p (t g h) -> p t g h", g=2, h=8)
        ksq, vsq = sqredv[:, :, 0, :], sqredv[:, :, 1, :]
        ve = small_pool.tile([128, NT * 8], F32, tag="ve")
        nc.vector.tensor_scalar_add(
            ve[:].rearrange("p (t h) -> p t h", h=8), vsq, D * EPS)
        prod = small_pool.tile([128, NT * 8], F32, tag="prod")
        nc.vector.scalar_tensor_tensor(
            prod[:].rearrange("p (t h) -> p t h", h=8),
            ksq, D * EPS, ve[:].rearrange("p (t h) -> p t h", h=8),
            op0=OP.add, op1=OP.mult)
        nc.scalar.activation(prod[:], prod[:], AF.Sqrt, scale=1.0 / (D * D))
        wsc = small_pool.tile([128, NT * 8], F32, tag="wsc")
        nc.vector.reciprocal(wsc[:], prod[:])
        w16 = small_pool.tile([128, NT * 8], F16, tag="w16")
        nc.vector.tensor_copy(w16[:], wsc[:])
        w16v = w16.rearrange("p (t h) -> p t h", h=8)

        # kt = kr * w  (DVE; the gpsimd queue must stay clear for collectives,
        # whose instruction blocks that queue until the collective completes)
        for nt in range(NT):
            kt = kt_pool.tile([128, 512], F16, tag="kt")
            kt_all[(b, nt)] = kt
            nc.vector.tensor_tensor(
                kt[:].rearrange("p (h d) -> p h d", d=D),
                kraw_all[(b, nt)][:].rearrange("p (h d) -> p h d", d=D),
                w16v[:, nt, :].broadcast_to([128, 8, D]),
                op=OP.mult)

        # dots: per head pair, accumulate kt^T vt over row tiles. Dots stay
        # UNSCALED (values ~O(100), f16-safe); 1/N folds into the W3 copy.
        bcols = NPAIR * 64
        d16 = d16_pool.tile([128, bcols], F16, tag="d16", name=f"d16_{b}")
        dots16_all[b] = d16
        for p in range(NPAIR):
            acc = dots_ps.tile([128, 128], F32, tag="t")
            for nt in range(NT):
                nc.tensor.matmul(
                    acc[:],
                    kt_all[(b, nt)][:, p * 128:(p + 1) * 128],
                    vt_all[(b, nt)][:, p * 128:(p + 1) * 128],
                    start=(nt == 0), stop=(nt == NT - 1))
            col = p * 64
            nc.scalar.copy(d16[0:64, col:col + 64], acc[0:64, 0:64])
            nc.scalar.copy(d16[64:128, col:col + 64], acc[64:128, 64:128])

        cc_in = dram.tile([128, bcols], F16, tag="cc_in", bufs=B, name=f"cc_in{b}")
        cc_out_all[b] = dram.tile([128, bcols], F16, tag="cc_out", bufs=B,
                                  name=f"cc_out{b}", addr_space="Shared")
        nc.sync.dma_start(cc_in[:], d16[:])
        nc.gpsimd.collective_compute(
            "AllReduce", OP.add,
            replica_groups=[list(range(ncores))],
            ins=[cc_in.opt()], outs=[cc_out_all[b].opt()])

    # ---------------- phase 2: compose W3 = Wq'^T dots' Wo^T, out = x W3 + bo ----
    def phase2(b):
        # fetch allreduced dots lazily (so the wait never head-of-line blocks
        # the staging of later batches' collectives)
        da = d16_pool.tile([128, NPAIR * 64], F16, tag="da", name=f"da_{b}")
        nc.sync.dma_start(da[:], cc_out_all[b][:])
        # W2T[j] [128i, 256c]: per head h, W2T rows h*64+e = dots_h^T @ Wq_h
        w2t16 = []
        for j in range(4):
            ps = big_ps.tile([128, 512], F32, tag="t")
            co = j * 64
            nc.tensor.matmul(ps[0:64, :CIN], da[0:64, co:co + 64],
                             wq_nat[j][0:64, :], start=True, stop=True)
            nc.tensor.matmul(ps[64:128, :CIN], da[64:128, co:co + 64],
                             wq_nat[j][64:128, :], start=True, stop=True)
            w2 = w2_pool.tile([128, CIN], F16, tag="w2")
            nc.scalar.copy(w2[:], ps[:, :CIN])
            w2t16.append(w2)
        # W3[cs] [128c, 256o] = sum_j W2T[j][:, cs]^T @ WoT[j]; 1/N lands here
        w3_16 = []
        for cs in range(2):
            ps = big_ps.tile([128, 512], F32, tag="t")
            for j in range(4):
                nc.tensor.matmul(ps[:, :COUT],
                                 w2t16[j][:, cs * 128:(cs + 1) * 128],
                                 wo_t[j][:], start=(j == 0), stop=(j == 3))
            w3 = w2_pool.tile([128, COUT], F16, tag="w3")
            nc.scalar.activation(w3[:], ps[:, :COUT], AF.Copy, scale=W3_SCALE)
            w3_16.append(w3)
        # out rows: out = xT^T @ W3 + bias (bias added in the psum->sbuf copy)
        for nt in range(NT):
            ops = big_ps.tile([128, 512], F32, tag="t")
            for cs in range(2):
                nc.tensor.matmul(ops[:, :COUT],
                                 xT_all[(b, cs)][:, nt * 128:(nt + 1) * 128],
                                 w3_16[cs][:], start=(cs == 0), stop=(cs == 1))
            osb = out_pool.tile([128, COUT], F32, tag="osb")
            nc.vector.tensor_tensor(osb[:], ops[:, :COUT], bias_bc[:], op=OP.add)
            nc.sync.dma_start(out_d[b, nt * 128:(nt + 1) * 128, :], osb[:])

    # schedule: PE order xp0 kv0 | xp1 dots0 kv1 | xp2 dots1 kv2 | ... so the
    # next batch's transposes fill the gap while stats/kt for batch b finish.
    xposes(0)
    kvstats(0)
    for b in range(B):
        if b + 1 < B:
            xposes(b + 1)
        ktdots(b)
        if b + 1 < B:
            kvstats(b + 1)
    for b in range(B):
        phase2(b)


_NC_CACHE = {}


def _get_nc(n_chunk, n_full, ncores):
    key = (n_chunk, n_full, ncores)
    if key not in _NC_CACHE:
        _NC_CACHE[key] = _build(n_chunk, n_full, ncores)
    return _NC_CACHE[key]


def _make_in_maps(u_x, Wq, Wk, Wv, Wo, bo, ncores):
    n = u_x.shape[1]
    n_chunk = n // ncores
    wq = np.ascontiguousarray(np.asarray(Wq, np.float32))
    wk = np.ascontiguousarray(np.asarray(Wk, np.float32))
    wv = np.ascontiguousarray(np.asarray(Wv, np.float32))
    wo = np.ascontiguousarray(np.asarray(Wo, np.float32))
    bo2 = np.ascontiguousarray(np.asarray(bo, np.float32).reshape(1, -1))
    u_x = np.asarray(u_x, np.float32)
    maps = []
    for c in range(ncores):
        maps.append({
            "x": np.ascontiguousarray(u_x[:, c * n_chunk:(c + 1) * n_chunk, :]),
            "wq": wq, "wk": wk, "wv": wv, "wo": wo, "bo": bo2,
        })
    return maps, n_chunk


def _install_ntff_hook():
    """Provide antenv.axon_hooks (missing in this image) so trace=True works."""
    import types
    try:
        from antenv.axon_hooks import get_axon_ntff_profile_hook  # noqa: F401
        return  # real module present
    except ImportError:
        pass
    try:
        import antenv
        mod = types.ModuleType("antenv.axon_hooks")
        _state = {"hook": None}
        mod.set_axon_ntff_profile_hook = lambda h: _state.__setitem__("hook", h)
        mod.get_axon_ntff_profile_hook = lambda: _state["hook"]
        sys.modules["antenv.axon_hooks"] = mod
        antenv.axon_hooks = mod
        boot_dir = "/root/.axon_site/trn_agent_boot"
        if boot_dir not in sys.path and os.path.isdir(boot_dir):
            sys.path.insert(0, boot_dir)
        import trn_boot
        so_path = "/opt/axon/libaxon_pjrt.so"
        if os.path.exists(so_path):
            hook = trn_boot._ntff_profile_via_ctypes(so_path)
            if hook is not None:
                mod.set_axon_ntff_profile_hook(hook)
    except Exception as e:  # tracing is best-effort; never break the run path
        print(f"ntff hook install failed: {e}", file=sys.stderr)


def run(u_x, Wq, Wk, Wv, Wo, bo, n_full=None, ncores=NCORES, trace=False,
        tmpdir=None):
    if trace:
        _install_ntff_hook()
    n = u_x.shape[1]
    if n_full is None:
        n_full = n
    in_maps, n_chunk = _make_in_maps(u_x, Wq, Wk, Wv, Wo, bo, ncores)
    nc = _get_nc(n_chunk, n_full, ncores)
    res = run_bass_kernel_spmd(nc, in_maps, list(range(ncores)), trace=trace,
                               tmpdir=tmpdir)
    outs = [np.asarray(res.results[c]["out"]) for c in range(ncores)]
    full = np.concatenate(outs, axis=1).astype(np.float32)
    return full, res


def kernel(u_x, pos_x=None, Wq=None, Wk=None, Wv=None, Wo=None, bo=None):
    full, _ = run(np.asarray(u_x, np.float32), Wq, Wk, Wv, Wo, bo)
    return full
